# revision 2
# baseline (speedup 1.0000x reference)
"""Expert-router (MoE top-2 routing) Trainium2 Bass kernel, 8-core SPMD.

Reference computation (fp32):
  logits[b,s,e] = hidden_states[b,s,:] @ W[e,:] + b[e] + routing_bias[b,e]
  p = softmax(logits, axis=-1)
  top2_vals, top2_idx = top_k(p, 2)
  weights = top2_vals / (sum(top2_vals) + 1e-8)
returns (weights [4,4096,2] f32, indices [4,4096,2] int32).

Sharding (data/sequence parallel): the 4*4096 = 16384 tokens are split
contiguously across 8 NeuronCores (2048 tokens each; every core's range
falls inside a single batch, so its routing_bias row is folded into the
bias vector on the host).  W^T and the bias are replicated.

The host pre-transposes each core's token shard to ht [4096, 2048]
(d-major), so the device kernel needs NO PE transposes (which in fp32
cost as much Tensor-engine time as the matmuls themselves).  Per core
(all fp32 — float32r matmul modes round operands to ~bf16 precision
and flip near-tie top-2 indices):

  - stream ht in 256-token blocks (chunk-split DMAs so the first
    matmuls start before the whole block lands)
  - per 128-token group: PSUM [128 tok, 64 exp] accumulates 32 d-chunk
    matmuls (lhsT = ht chunk stationary, rhs = W^T chunk moving) on top
    of a rank-1 ones x (b + routing_bias) matmul that folds the bias in
  - top-8 / softmax / renorm run straight off PSUM: DVE max/max_index,
    ACT exp with accumulated Z, exact DVE reciprocals:
        p1 = 1/Z, p2 = exp(l2-l1)/Z, w_i = p_i/(p1+p2+1e-8)
  - outputs accumulate in SBUF [128, 16, 2] and stream out per block
"""

from contextlib import ExitStack

import numpy as np

import concourse.mybir as mybir
import concourse.tile as tile
from concourse import bacc
from concourse.bass_utils import run_bass_kernel_spmd

P = 128
D = 4096
E = 64
NCHUNK = D // P          # 32
N_CORES = 8
B_DIM = 4
S = 4096                 # tokens per batch
T_TOTAL = B_DIM * S      # 16384
T_CORE = T_TOTAL // N_CORES  # 2048
TOKBLK = 256             # tokens per DMA/compute block
DMA_SPLIT = 4            # chunk-range pieces per block DMA
EPS = 1e-8

f32 = mybir.dt.float32
i32 = mybir.dt.int32
u32 = mybir.dt.uint32

_NC_CACHE = None


def _build_nc():
    nblk = T_CORE // TOKBLK      # 8 blocks per core
    gp = TOKBLK // P             # 2 groups per block
    ngrp = T_CORE // P           # 16
    cs = NCHUNK // DMA_SPLIT

    nc = bacc.Bacc("TRN2", target_bir_lowering=False, debug=False)

    ht = nc.dram_tensor("ht", [D, T_CORE], f32, kind="ExternalInput").ap()
    wt = nc.dram_tensor("wt", [D, E], f32, kind="ExternalInput").ap()
    bias = nc.dram_tensor("bias", [1, E], f32, kind="ExternalInput").ap()
    ones = nc.dram_tensor("ones", [1, P], f32, kind="ExternalInput").ap()
    w_out = nc.dram_tensor("w_out", [P, ngrp, 2], f32, kind="ExternalOutput").ap()
    i_out = nc.dram_tensor("i_out", [P, ngrp, 2], i32, kind="ExternalOutput").ap()

    with tile.TileContext(nc) as tc, ExitStack() as ctx:
        const = ctx.enter_context(tc.tile_pool(name="const", bufs=1))
        wt_sb = const.tile([P, NCHUNK, E], f32)
        nc.sync.dma_start(wt_sb[:], wt.rearrange("(c p) e -> p c e", p=P))
        bias_sb = const.tile([1, E], f32)
        nc.sync.dma_start(bias_sb[:], bias)
        ones_sb = const.tile([1, P], f32)
        nc.sync.dma_start(ones_sb[:], ones)
        w_all = const.tile([P, ngrp, 2], f32)
        i_all = const.tile([P, ngrp, 2], i32)

        in_pool = ctx.enter_context(tc.tile_pool(name="hin", bufs=3))
        sm_pool = ctx.enter_context(tc.tile_pool(name="sm", bufs=3))
        mm_psum = ctx.enter_context(tc.tile_pool(name="mm_psum", bufs=6, space="PSUM"))

        for blk in range(nblk):
            t0 = blk * TOKBLK
            ht_sb = in_pool.tile([P, NCHUNK, TOKBLK], f32, tag="hin")
            for piece in range(DMA_SPLIT):
                c0 = piece * cs
                nc.sync.dma_start(
                    ht_sb[:, c0 : c0 + cs, :],
                    ht[c0 * P : (c0 + cs) * P, t0 : t0 + TOKBLK].rearrange(
                        "(c p) t -> p c t", p=P
                    ),
                )

            for g in range(gp):
                grp = blk * gp + g
                lg = mm_psum.tile([P, E], f32, tag="mm")
                # PSUM = ones^T @ (b + routing_bias): exact fp32 bias
                # preload, then accumulate the 32 d-chunks on top.
                nc.tensor.matmul(
                    lg[:], lhsT=ones_sb[:], rhs=bias_sb[:], start=True, stop=False
                )
                for c in range(NCHUNK):
                    nc.tensor.matmul(
                        lg[:],
                        lhsT=ht_sb[:, c, g * P : (g + 1) * P],
                        rhs=wt_sb[:, c],
                        start=False,
                        stop=(c == NCHUNK - 1),
                    )

                mx8 = sm_pool.tile([P, 8], f32, tag="mx8")
                nc.vector.max(out=mx8[:], in_=lg[:])
                idx8 = sm_pool.tile([P, 8], u32, tag="idx8")
                nc.vector.max_index(idx8[:], mx8[:], lg[:])
                nc.vector.tensor_copy(i_all[:, grp, :], idx8[:, 0:2])

                negm = sm_pool.tile([P, 1], f32, tag="negm")
                nc.vector.tensor_scalar_mul(negm[:], mx8[:, 0:1], -1.0)
                escr = sm_pool.tile([P, E], f32, tag="escr")
                zsum = sm_pool.tile([P, 1], f32, tag="zsum")
                nc.scalar.activation(
                    escr[:],
                    lg[:],
                    mybir.ActivationFunctionType.Exp,
                    bias=negm[:],
                    scale=1.0,
                    accum_out=zsum[:],
                )
                e2 = sm_pool.tile([P, 1], f32, tag="e2")
                nc.scalar.activation(
                    e2[:],
                    mx8[:, 1:2],
                    mybir.ActivationFunctionType.Exp,
                    bias=negm[:],
                    scale=1.0,
                )
                zr = sm_pool.tile([P, 1], f32, tag="zr")
                nc.vector.reciprocal(zr[:], zsum[:])
                p2 = sm_pool.tile([P, 1], f32, tag="p2")
                nc.vector.tensor_mul(p2[:], e2[:], zr[:])
                s = sm_pool.tile([P, 1], f32, tag="s")
                nc.vector.tensor_add(s[:], zr[:], p2[:])
                nc.vector.tensor_scalar_add(s[:], s[:], EPS)
                sr = sm_pool.tile([P, 1], f32, tag="sr")
                nc.vector.reciprocal(sr[:], s[:])
                nc.vector.tensor_mul(w_all[:, grp, 0:1], zr[:], sr[:])
                nc.vector.tensor_mul(w_all[:, grp, 1:2], p2[:], sr[:])

            gsl = slice(blk * gp, (blk + 1) * gp)
            nc.sync.dma_start(w_out[:, gsl, :], w_all[:, gsl, :])
            nc.sync.dma_start(i_out[:, gsl, :], i_all[:, gsl, :])

    nc.compile()
    return nc


def _get_nc():
    global _NC_CACHE
    if _NC_CACHE is None:
        _NC_CACHE = _build_nc()
    return _NC_CACHE


def kernel(hidden_states, routing_bias, W, b):
    h2 = np.ascontiguousarray(np.asarray(hidden_states, dtype=np.float32)).reshape(
        T_TOTAL, D
    )
    wt = np.ascontiguousarray(np.asarray(W, dtype=np.float32).T)
    bnp = np.asarray(b, dtype=np.float32)
    rb = np.asarray(routing_bias, dtype=np.float32)
    ones = np.ones((1, P), dtype=np.float32)

    in_maps = []
    for c in range(N_CORES):
        t0 = c * T_CORE
        batch = t0 // S  # each core's token range lies within one batch
        in_maps.append(
            {
                "ht": np.ascontiguousarray(h2[t0 : t0 + T_CORE].T),
                "wt": wt,
                "bias": (bnp + rb[batch]).astype(np.float32).reshape(1, E),
                "ones": ones,
            }
        )

    nc = _get_nc()
    try:
        res = run_bass_kernel_spmd(nc, in_maps, list(range(N_CORES)))
    except Exception:
        # transient NRT/device hiccups have been observed to clear on retry
        res = run_bass_kernel_spmd(nc, in_maps, list(range(N_CORES)))

    ws, idxs = [], []
    for r in res.results:
        # [P, ngrp, 2] with token = grp*128 + partition
        ws.append(np.asarray(r["w_out"]).transpose(1, 0, 2).reshape(T_CORE, 2))
        idxs.append(np.asarray(r["i_out"]).transpose(1, 0, 2).reshape(T_CORE, 2))
    weights = np.concatenate(ws).reshape(B_DIM, S, 2).astype(np.float32)
    indices = np.concatenate(idxs).reshape(B_DIM, S, 2).astype(np.int32)
    return weights, indices


# revision 3
# speedup vs baseline: 1.7320x; 1.7320x over previous
"""Expert-router (MoE top-2 routing) Trainium2 Bass kernel, 8-core SPMD.

Reference computation (fp32):
  logits[b,s,e] = hidden_states[b,s,:] @ W[e,:] + b[e] + routing_bias[b,e]
  p = softmax(logits, axis=-1)
  top2_vals, top2_idx = top_k(p, 2)
  weights = top2_vals / (sum(top2_vals) + 1e-8)
returns (weights [4,4096,2] f32, indices [4,4096,2] int32).

Sharding (data/sequence parallel): the 4*4096 = 16384 tokens are split
contiguously across 8 NeuronCores (2048 tokens each; every core's range
falls inside a single batch, so its routing_bias row is folded into the
bias vector on the host).  W^T and the bias are replicated.

Numerics (scaled fp16 hi/lo): fp32 matmuls measure ~436 ns on this PE
(multi-pass streaming + slow fp32 LD_WEIGHTS) while half-word matmuls
of the same shape measure ~40 ns.  So the host splits the operands into
fp16 hi + fp16 residual parts whose 3-term product reproduces the fp32
logits to ~2^-22 relative (fp32-class accuracy; top-2 ranking and
softmax weights match the fp32 reference — verified 0/32768 index
mismatches):

  h      = a + b          (a = fp16(h), b = fp16(h - a))
  W^T*32 = u + v          (scaling by 2^5 keeps v in fp16 normal range)
  32*logits = a@u + a@v + b@u  (+ ones x 32*(bias+routing_bias) hi/lo)

The dropped b@v term is O(2^-22 |h||W|).  Softmax runs off the scaled
logits via ACT's scale=1/32 hook (exact power of two), changing neither
ranking nor softmax values.

Per core the host pre-transposes the token shard to a/b [4096, 2048]
fp16 (d-major), so no PE transposes are needed.  The kernel streams
512-token blocks (chunk-split DMAs so matmuls start before the whole
block lands); per 128-token group PSUM [128 tok, 64 exp] accumulates
2 rank-1 bias matmuls + 32 d-chunks x 3 fp16 matmuls; top-8 / softmax /
renorm (DVE max/max_index, ACT exp with accumulated Z, exact DVE
reciprocals) read PSUM directly:
    p1 = 1/Z, p2 = exp((l2-l1)/32)/Z, w_i = p_i/(p1+p2+1e-8)
"""

from contextlib import ExitStack

import numpy as np

import concourse.mybir as mybir
import concourse.tile as tile
from concourse import bacc
from concourse.bass_utils import run_bass_kernel_spmd

P = 128
D = 4096
E = 64
NCHUNK = D // P          # 32
N_CORES = 8
B_DIM = 4
S = 4096                 # tokens per batch
T_TOTAL = B_DIM * S      # 16384
T_CORE = T_TOTAL // N_CORES  # 2048
TOKBLK = 512             # tokens per DMA/compute block
IN_BUFS = 2
DMA_SPLIT = 4            # chunk-range pieces per block DMA (per hi/lo)
WSCALE = 32.0            # power of 2: exact, ranking-preserving
EPS = 1e-8

f32 = mybir.dt.float32
f16 = mybir.dt.float16
i32 = mybir.dt.int32
u32 = mybir.dt.uint32

_NC_CACHE = None


def _build_nc():
    nblk = T_CORE // TOKBLK      # 4 blocks per core
    gp = TOKBLK // P             # 4 groups per block
    ngrp = T_CORE // P           # 16
    cs = NCHUNK // DMA_SPLIT

    nc = bacc.Bacc("TRN2", target_bir_lowering=False, debug=False)

    ht_hi = nc.dram_tensor("ht_hi", [D, T_CORE], f16, kind="ExternalInput").ap()
    ht_lo = nc.dram_tensor("ht_lo", [D, T_CORE], f16, kind="ExternalInput").ap()
    wt_hi = nc.dram_tensor("wt_hi", [D, E], f16, kind="ExternalInput").ap()
    wt_lo = nc.dram_tensor("wt_lo", [D, E], f16, kind="ExternalInput").ap()
    bias_hl = nc.dram_tensor("bias_hl", [1, 2 * E], f16, kind="ExternalInput").ap()
    ones = nc.dram_tensor("ones", [1, P], f16, kind="ExternalInput").ap()
    w_out = nc.dram_tensor("w_out", [P, ngrp, 2], f32, kind="ExternalOutput").ap()
    i_out = nc.dram_tensor("i_out", [P, ngrp, 2], i32, kind="ExternalOutput").ap()

    with tile.TileContext(nc) as tc, ExitStack() as ctx:
        const = ctx.enter_context(tc.tile_pool(name="const", bufs=1))
        wu_sb = const.tile([P, NCHUNK, E], f16)
        nc.sync.dma_start(wu_sb[:], wt_hi.rearrange("(c p) e -> p c e", p=P))
        wv_sb = const.tile([P, NCHUNK, E], f16)
        nc.sync.dma_start(wv_sb[:], wt_lo.rearrange("(c p) e -> p c e", p=P))
        bias_sb = const.tile([1, 2 * E], f16)
        nc.sync.dma_start(bias_sb[:], bias_hl)
        ones_sb = const.tile([1, P], f16)
        nc.sync.dma_start(ones_sb[:], ones)
        w_all = const.tile([P, ngrp, 2], f32)
        i_all = const.tile([P, ngrp, 2], i32)

        in_pool = ctx.enter_context(tc.tile_pool(name="hin", bufs=IN_BUFS))
        sm_pool = ctx.enter_context(tc.tile_pool(name="sm", bufs=3))
        mm_psum = ctx.enter_context(tc.tile_pool(name="mm_psum", bufs=6, space="PSUM"))

        for blk in range(nblk):
            t0 = blk * TOKBLK
            a_sb = in_pool.tile([P, NCHUNK, TOKBLK], f16, tag="ahin")
            b_sb = in_pool.tile([P, NCHUNK, TOKBLK], f16, tag="bhin")
            for piece in range(DMA_SPLIT):
                c0 = piece * cs
                nc.sync.dma_start(
                    a_sb[:, c0 : c0 + cs, :],
                    ht_hi[c0 * P : (c0 + cs) * P, t0 : t0 + TOKBLK].rearrange(
                        "(c p) t -> p c t", p=P
                    ),
                )
                nc.sync.dma_start(
                    b_sb[:, c0 : c0 + cs, :],
                    ht_lo[c0 * P : (c0 + cs) * P, t0 : t0 + TOKBLK].rearrange(
                        "(c p) t -> p c t", p=P
                    ),
                )

            for g in range(gp):
                grp = blk * gp + g
                gs = slice(g * P, (g + 1) * P)
                lg = mm_psum.tile([P, E], f32, tag="mm")
                # exact scaled bias preload: ones^T @ (32*(b+rb))_hi/lo
                nc.tensor.matmul(
                    lg[:], lhsT=ones_sb[:], rhs=bias_sb[:, 0:E],
                    start=True, stop=False,
                )
                nc.tensor.matmul(
                    lg[:], lhsT=ones_sb[:], rhs=bias_sb[:, E : 2 * E],
                    start=False, stop=False,
                )
                for c in range(NCHUNK):
                    nc.tensor.matmul(
                        lg[:], lhsT=a_sb[:, c, gs], rhs=wu_sb[:, c],
                        start=False, stop=False,
                    )
                    nc.tensor.matmul(
                        lg[:], lhsT=a_sb[:, c, gs], rhs=wv_sb[:, c],
                        start=False, stop=False,
                    )
                    nc.tensor.matmul(
                        lg[:], lhsT=b_sb[:, c, gs], rhs=wu_sb[:, c],
                        start=False, stop=(c == NCHUNK - 1),
                    )

                mx8 = sm_pool.tile([P, 8], f32, tag="mx8")
                nc.vector.max(out=mx8[:], in_=lg[:])
                idx8 = sm_pool.tile([P, 8], u32, tag="idx8")
                nc.vector.max_index(idx8[:], mx8[:], lg[:])
                nc.vector.tensor_copy(i_all[:, grp, :], idx8[:, 0:2])

                # scaled-logit softmax: exp(x/WSCALE + bias), so negm folds
                # the -max/WSCALE shift in one DVE op
                negm = sm_pool.tile([P, 1], f32, tag="negm")
                nc.vector.tensor_scalar_mul(negm[:], mx8[:, 0:1], -1.0 / WSCALE)
                escr = sm_pool.tile([P, E], f32, tag="escr")
                zsum = sm_pool.tile([P, 1], f32, tag="zsum")
                nc.scalar.activation(
                    escr[:],
                    lg[:],
                    mybir.ActivationFunctionType.Exp,
                    bias=negm[:],
                    scale=1.0 / WSCALE,
                    accum_out=zsum[:],
                )
                e2 = sm_pool.tile([P, 1], f32, tag="e2")
                nc.scalar.activation(
                    e2[:],
                    mx8[:, 1:2],
                    mybir.ActivationFunctionType.Exp,
                    bias=negm[:],
                    scale=1.0 / WSCALE,
                )
                zr = sm_pool.tile([P, 1], f32, tag="zr")
                nc.vector.reciprocal(zr[:], zsum[:])
                p2 = sm_pool.tile([P, 1], f32, tag="p2")
                nc.vector.tensor_mul(p2[:], e2[:], zr[:])
                s = sm_pool.tile([P, 1], f32, tag="s")
                nc.vector.tensor_add(s[:], zr[:], p2[:])
                nc.vector.tensor_scalar_add(s[:], s[:], EPS)
                sr = sm_pool.tile([P, 1], f32, tag="sr")
                nc.vector.reciprocal(sr[:], s[:])
                nc.vector.tensor_mul(w_all[:, grp, 0:1], zr[:], sr[:])
                nc.vector.tensor_mul(w_all[:, grp, 1:2], p2[:], sr[:])

            gsl = slice(blk * gp, (blk + 1) * gp)
            nc.sync.dma_start(w_out[:, gsl, :], w_all[:, gsl, :])
            nc.sync.dma_start(i_out[:, gsl, :], i_all[:, gsl, :])

    nc.compile()
    return nc


def _get_nc():
    global _NC_CACHE
    if _NC_CACHE is None:
        _NC_CACHE = _build_nc()
    return _NC_CACHE


def _split_fp16(x):
    hi = x.astype(np.float16)
    lo = (x - hi.astype(np.float32)).astype(np.float16)
    return hi, lo


def kernel(hidden_states, routing_bias, W, b):
    h2 = np.ascontiguousarray(np.asarray(hidden_states, dtype=np.float32)).reshape(
        T_TOTAL, D
    )
    wt32 = np.asarray(W, dtype=np.float32).T * WSCALE
    wt_hi, wt_lo = _split_fp16(wt32)
    wt_hi = np.ascontiguousarray(wt_hi)
    wt_lo = np.ascontiguousarray(wt_lo)
    bnp = np.asarray(b, dtype=np.float32)
    rb = np.asarray(routing_bias, dtype=np.float32)
    ones = np.ones((1, P), dtype=np.float16)

    in_maps = []
    for c in range(N_CORES):
        t0 = c * T_CORE
        batch = t0 // S  # each core's token range lies within one batch
        ht = np.ascontiguousarray(h2[t0 : t0 + T_CORE].T)
        ht_hi, ht_lo = _split_fp16(ht)
        bias32 = ((bnp + rb[batch]) * WSCALE).astype(np.float32)
        b_hi, b_lo = _split_fp16(bias32)
        in_maps.append(
            {
                "ht_hi": np.ascontiguousarray(ht_hi),
                "ht_lo": np.ascontiguousarray(ht_lo),
                "wt_hi": wt_hi,
                "wt_lo": wt_lo,
                "bias_hl": np.concatenate([b_hi, b_lo]).reshape(1, 2 * E),
                "ones": ones,
            }
        )

    nc = _get_nc()
    try:
        res = run_bass_kernel_spmd(nc, in_maps, list(range(N_CORES)))
    except Exception:
        # transient NRT/device hiccups have been observed to clear on retry
        res = run_bass_kernel_spmd(nc, in_maps, list(range(N_CORES)))

    ws, idxs = [], []
    for r in res.results:
        # [P, ngrp, 2] with token = grp*128 + partition
        ws.append(np.asarray(r["w_out"]).transpose(1, 0, 2).reshape(T_CORE, 2))
        idxs.append(np.asarray(r["i_out"]).transpose(1, 0, 2).reshape(T_CORE, 2))
    weights = np.concatenate(ws).reshape(B_DIM, S, 2).astype(np.float32)
    indices = np.concatenate(idxs).reshape(B_DIM, S, 2).astype(np.int32)
    return weights, indices


# revision 7
# speedup vs baseline: 1.8586x; 1.0731x over previous
"""Expert-router (MoE top-2 routing) Trainium2 Bass kernel, 8-core SPMD.

Reference computation (fp32):
  logits[b,s,e] = hidden_states[b,s,:] @ W[e,:] + b[e] + routing_bias[b,e]
  p = softmax(logits, axis=-1)
  top2_vals, top2_idx = top_k(p, 2)
  weights = top2_vals / (sum(top2_vals) + 1e-8)
returns (weights [4,4096,2] f32, indices [4,4096,2] int32).

Sharding (data/sequence parallel): the 4*4096 = 16384 tokens are split
contiguously across 8 NeuronCores (2048 tokens each; every core's range
falls inside a single batch, so its routing_bias row is folded into the
bias vector on the host).  W^T and the bias are replicated.

Numerics (scaled fp16 hi/lo): fp32 matmuls measure ~436 ns on this PE
(multi-pass streaming + slow fp32 LD_WEIGHTS) while half-word matmuls
of the same shape measure ~40 ns.  So the host splits the operands into
fp16 hi + fp16 residual parts whose 3-term product reproduces the fp32
logits to ~2^-22 relative (fp32-class accuracy; top-2 ranking and
softmax weights match the fp32 reference — verified 0/32768 index
mismatches):

  h      = a + b          (a = fp16(h), b = fp16(h - a))
  W^T*32 = u + v          (scaling by 2^5 keeps v in fp16 normal range)
  32*logits = a@u + a@v + b@u  (+ ones x 32*(bias+routing_bias) hi/lo)

The dropped b@v term is O(2^-22 |h||W|).  Softmax runs off the scaled
logits via ACT's scale=1/32 hook (exact power of two), changing neither
ranking nor softmax values.

Per core the host pre-transposes the token shard to a/b [4096, 2048]
fp16 (d-major), so no PE transposes are needed.  The kernel streams
512-token blocks (chunk-split DMAs so matmuls start before the whole
block lands); per 128-token group PSUM [128 tok, 64 exp] accumulates
2 rank-1 bias matmuls + 32 d-chunks x 3 fp16 matmuls; top-8 / softmax /
renorm (DVE max/max_index, ACT exp with accumulated Z, exact DVE
reciprocals) read PSUM directly:
    p1 = 1/Z, p2 = exp((l2-l1)/32)/Z, w_i = p_i/(p1+p2+1e-8)
"""

from contextlib import ExitStack

import numpy as np

import concourse.mybir as mybir
import concourse.tile as tile
from concourse import bacc
from concourse.bass_utils import run_bass_kernel_spmd

P = 128
D = 4096
E = 64
NCHUNK = D // P          # 32
N_CORES = 8
B_DIM = 4
S = 4096                 # tokens per batch
T_TOTAL = B_DIM * S      # 16384
T_CORE = T_TOTAL // N_CORES  # 2048
TOKBLK = 256             # tokens per DMA/compute block
IN_BUFS = 4
DMA_SPLIT = 4            # chunk-range pieces per block DMA (per hi/lo)
WSCALE = 32.0            # power of 2: exact, ranking-preserving
EPS = 1e-8

f32 = mybir.dt.float32
f16 = mybir.dt.float16
i32 = mybir.dt.int32
u32 = mybir.dt.uint32

_NC_CACHE = None


def _build_nc():
    nblk = T_CORE // TOKBLK      # 4 blocks per core
    gp = TOKBLK // P             # 4 groups per block
    ngrp = T_CORE // P           # 16
    cs = NCHUNK // DMA_SPLIT

    nc = bacc.Bacc("TRN2", target_bir_lowering=False, debug=False)

    # hi/lo shards pre-laid-out on the host as the SBUF block image:
    # [blk][p][c*TOKBLK+t] = h[blk*TOKBLK+t, c*128+p] -> per-partition
    # contiguous DMA lines
    ht_hi = nc.dram_tensor(
        "ht_hi", [nblk, P, NCHUNK * TOKBLK], f16, kind="ExternalInput").ap()
    ht_lo = nc.dram_tensor(
        "ht_lo", [nblk, P, NCHUNK * TOKBLK], f16, kind="ExternalInput").ap()
    wt_hi = nc.dram_tensor("wt_hi", [D, E], f16, kind="ExternalInput").ap()
    wt_lo = nc.dram_tensor("wt_lo", [D, E], f16, kind="ExternalInput").ap()
    bias_hl = nc.dram_tensor("bias_hl", [1, 2 * E], f16, kind="ExternalInput").ap()
    ones = nc.dram_tensor("ones", [1, P], f16, kind="ExternalInput").ap()
    w_out = nc.dram_tensor("w_out", [P, ngrp, 2], f32, kind="ExternalOutput").ap()
    i_out = nc.dram_tensor("i_out", [P, ngrp, 2], i32, kind="ExternalOutput").ap()

    with tile.TileContext(nc) as tc, ExitStack() as ctx:
        const = ctx.enter_context(tc.tile_pool(name="const", bufs=1))
        wu_sb = const.tile([P, NCHUNK, E], f16)
        nc.sync.dma_start(wu_sb[:], wt_hi.rearrange("(c p) e -> p c e", p=P))
        wv_sb = const.tile([P, NCHUNK, E], f16)
        nc.sync.dma_start(wv_sb[:], wt_lo.rearrange("(c p) e -> p c e", p=P))
        bias_sb = const.tile([1, 2 * E], f16)
        nc.sync.dma_start(bias_sb[:], bias_hl)
        ones_sb = const.tile([1, P], f16)
        nc.sync.dma_start(ones_sb[:], ones)
        w_all = const.tile([P, ngrp, 2], f32)
        i_all = const.tile([P, ngrp, 2], i32)

        in_pool = ctx.enter_context(tc.tile_pool(name="hin", bufs=IN_BUFS))
        sm_pool = ctx.enter_context(tc.tile_pool(name="sm", bufs=3))
        mm_psum = ctx.enter_context(tc.tile_pool(name="mm_psum", bufs=6, space="PSUM"))

        for blk in range(nblk):
            t0 = blk * TOKBLK
            a_sb = in_pool.tile([P, NCHUNK, TOKBLK], f16, tag="ahin")
            b_sb = in_pool.tile([P, NCHUNK, TOKBLK], f16, tag="bhin")
            # lo-tensor DMAs issue from the Activation HWDGE queues so both
            # queue sets pull from HBM in parallel
            for piece in range(DMA_SPLIT):
                c0 = piece * cs
                fs = slice(c0 * TOKBLK, (c0 + cs) * TOKBLK)
                nc.sync.dma_start(a_sb[:, c0 : c0 + cs, :], ht_hi[blk][:, fs])
                nc.scalar.dma_start(b_sb[:, c0 : c0 + cs, :], ht_lo[blk][:, fs])

            for g in range(gp):
                grp = blk * gp + g
                gs = slice(g * P, (g + 1) * P)
                lg = mm_psum.tile([P, E], f32, tag="mm")
                # exact scaled bias preload: ones^T @ (32*(b+rb))_hi/lo
                nc.tensor.matmul(
                    lg[:], lhsT=ones_sb[:], rhs=bias_sb[:, 0:E],
                    start=True, stop=False,
                )
                nc.tensor.matmul(
                    lg[:], lhsT=ones_sb[:], rhs=bias_sb[:, E : 2 * E],
                    start=False, stop=False,
                )
                for c in range(NCHUNK):
                    nc.tensor.matmul(
                        lg[:], lhsT=a_sb[:, c, gs], rhs=wu_sb[:, c],
                        start=False, stop=False,
                    )
                    nc.tensor.matmul(
                        lg[:], lhsT=a_sb[:, c, gs], rhs=wv_sb[:, c],
                        start=False, stop=False,
                    )
                    nc.tensor.matmul(
                        lg[:], lhsT=b_sb[:, c, gs], rhs=wu_sb[:, c],
                        start=False, stop=(c == NCHUNK - 1),
                    )

                mx8 = sm_pool.tile([P, 8], f32, tag="mx8")
                nc.vector.max(out=mx8[:], in_=lg[:])
                idx8 = sm_pool.tile([P, 8], u32, tag="idx8")
                nc.vector.max_index(idx8[:], mx8[:], lg[:])
                nc.vector.tensor_copy(i_all[:, grp, :], idx8[:, 0:2])

                # scaled-logit softmax: exp(x/WSCALE + bias), so negm folds
                # the -max/WSCALE shift in one DVE op
                negm = sm_pool.tile([P, 1], f32, tag="negm")
                nc.vector.tensor_scalar_mul(negm[:], mx8[:, 0:1], -1.0 / WSCALE)
                escr = sm_pool.tile([P, E], f32, tag="escr")
                zsum = sm_pool.tile([P, 1], f32, tag="zsum")
                nc.scalar.activation(
                    escr[:],
                    lg[:],
                    mybir.ActivationFunctionType.Exp,
                    bias=negm[:],
                    scale=1.0 / WSCALE,
                    accum_out=zsum[:],
                )
                e2 = sm_pool.tile([P, 1], f32, tag="e2")
                nc.scalar.activation(
                    e2[:],
                    mx8[:, 1:2],
                    mybir.ActivationFunctionType.Exp,
                    bias=negm[:],
                    scale=1.0 / WSCALE,
                )
                zr = sm_pool.tile([P, 1], f32, tag="zr")
                nc.vector.reciprocal(zr[:], zsum[:])
                p2 = sm_pool.tile([P, 1], f32, tag="p2")
                nc.vector.tensor_mul(p2[:], e2[:], zr[:])
                s = sm_pool.tile([P, 1], f32, tag="s")
                nc.vector.tensor_add(s[:], zr[:], p2[:])
                nc.vector.tensor_scalar_add(s[:], s[:], EPS)
                sr = sm_pool.tile([P, 1], f32, tag="sr")
                nc.vector.reciprocal(sr[:], s[:])
                nc.vector.tensor_mul(w_all[:, grp, 0:1], zr[:], sr[:])
                nc.vector.tensor_mul(w_all[:, grp, 1:2], p2[:], sr[:])

            gsl = slice(blk * gp, (blk + 1) * gp)
            nc.sync.dma_start(w_out[:, gsl, :], w_all[:, gsl, :])
            nc.sync.dma_start(i_out[:, gsl, :], i_all[:, gsl, :])

    nc.compile()
    return nc


def _get_nc():
    global _NC_CACHE
    if _NC_CACHE is None:
        _NC_CACHE = _build_nc()
    return _NC_CACHE


def _split_fp16(x):
    hi = x.astype(np.float16)
    lo = (x - hi.astype(np.float32)).astype(np.float16)
    return hi, lo


def kernel(hidden_states, routing_bias, W, b):
    h2 = np.ascontiguousarray(np.asarray(hidden_states, dtype=np.float32)).reshape(
        T_TOTAL, D
    )
    wt32 = np.asarray(W, dtype=np.float32).T * WSCALE
    wt_hi, wt_lo = _split_fp16(wt32)
    wt_hi = np.ascontiguousarray(wt_hi)
    wt_lo = np.ascontiguousarray(wt_lo)
    bnp = np.asarray(b, dtype=np.float32)
    rb = np.asarray(routing_bias, dtype=np.float32)
    ones = np.ones((1, P), dtype=np.float16)

    nblk = T_CORE // TOKBLK

    def _blocked(x):
        # SBUF block image: [blk, p, c*TOKBLK+t] = h[blk*TOKBLK+t, c*128+p]
        return np.ascontiguousarray(
            x.reshape(nblk, TOKBLK, NCHUNK, P).transpose(0, 3, 2, 1)
        ).reshape(nblk, P, NCHUNK * TOKBLK)

    in_maps = []
    for c in range(N_CORES):
        t0 = c * T_CORE
        batch = t0 // S  # each core's token range lies within one batch
        h_hi, h_lo = _split_fp16(h2[t0 : t0 + T_CORE])
        bias32 = ((bnp + rb[batch]) * WSCALE).astype(np.float32)
        b_hi, b_lo = _split_fp16(bias32)
        in_maps.append(
            {
                "ht_hi": _blocked(h_hi),
                "ht_lo": _blocked(h_lo),
                "wt_hi": wt_hi,
                "wt_lo": wt_lo,
                "bias_hl": np.concatenate([b_hi, b_lo]).reshape(1, 2 * E),
                "ones": ones,
            }
        )

    nc = _get_nc()
    try:
        res = run_bass_kernel_spmd(nc, in_maps, list(range(N_CORES)))
    except Exception:
        # transient NRT/device hiccups have been observed to clear on retry
        res = run_bass_kernel_spmd(nc, in_maps, list(range(N_CORES)))

    ws, idxs = [], []
    for r in res.results:
        # [P, ngrp, 2] with token = grp*128 + partition
        ws.append(np.asarray(r["w_out"]).transpose(1, 0, 2).reshape(T_CORE, 2))
        idxs.append(np.asarray(r["i_out"]).transpose(1, 0, 2).reshape(T_CORE, 2))
    weights = np.concatenate(ws).reshape(B_DIM, S, 2).astype(np.float32)
    indices = np.concatenate(idxs).reshape(B_DIM, S, 2).astype(np.int32)
    return weights, indices


# revision 9
# speedup vs baseline: 1.9895x; 1.0705x over previous
"""Expert-router (MoE top-2 routing) Trainium2 Bass kernel, 8-core SPMD.

Reference computation (fp32):
  logits[b,s,e] = hidden_states[b,s,:] @ W[e,:] + b[e] + routing_bias[b,e]
  p = softmax(logits, axis=-1)
  top2_vals, top2_idx = top_k(p, 2)
  weights = top2_vals / (sum(top2_vals) + 1e-8)
returns (weights [4,4096,2] f32, indices [4,4096,2] int32).

Sharding (data/sequence parallel): the 4*4096 = 16384 tokens are split
contiguously across 8 NeuronCores (2048 tokens each; every core's range
falls inside a single batch, so its routing_bias row is folded into the
bias vector on the host).  W^T and the bias are replicated.

Numerics (scaled fp16 hi/lo): fp32 matmuls measure ~436 ns on this PE
(multi-pass streaming + slow fp32 LD_WEIGHTS) while half-word matmuls
of the same shape measure ~40 ns.  So the host splits the operands into
fp16 hi + fp16 residual parts whose 3-term product reproduces the fp32
logits to ~2^-22 relative (fp32-class accuracy; top-2 ranking and
softmax weights match the fp32 reference — verified 0/32768 index
mismatches):

  h      = a + b          (a = fp16(h), b = fp16(h - a))
  W^T*32 = u + v          (scaling by 2^5 keeps v in fp16 normal range)
  32*logits = a@u + a@v + b@u  (+ ones x 32*(bias+routing_bias) hi/lo)

The dropped b@v term is O(2^-22 |h||W|).  Softmax runs off the scaled
logits via ACT's scale=1/32 hook (exact power of two), changing neither
ranking nor softmax values.

Per core the host pre-transposes the token shard to a/b [4096, 2048]
fp16 (d-major), so no PE transposes are needed.  The kernel streams
256-token blocks (chunk-split DMAs so matmuls start before the whole
block lands); per 128-token group PSUM [128 tok, 64 exp] accumulates
2 rank-1 bias matmuls + 32 d-chunks x 3 fp16 matmuls; top-8 / softmax /
renorm (DVE max/max_index, ACT exp with accumulated Z, exact DVE
reciprocals) read PSUM directly:
    p1 = 1/Z, p2 = exp((l2-l1)/32)/Z, w_i = p_i/(p1+p2+1e-8)
"""

from contextlib import ExitStack

import numpy as np

import concourse.mybir as mybir
import concourse.tile as tile
from concourse import bacc
from concourse.bass_utils import run_bass_kernel_spmd

P = 128
D = 4096
E = 64
NCHUNK = D // P          # 32
N_CORES = 8
B_DIM = 4
S = 4096                 # tokens per batch
T_TOTAL = B_DIM * S      # 16384
T_CORE = T_TOTAL // N_CORES  # 2048
TOKBLK = 256             # tokens per DMA/compute block
IN_BUFS = 5
DMA_SPLIT = 2            # chunk-range pieces per block DMA (per hi/lo)
WSCALE = 32.0            # power of 2: exact, ranking-preserving
EPS = 1e-8

f32 = mybir.dt.float32
f16 = mybir.dt.float16
i32 = mybir.dt.int32
u32 = mybir.dt.uint32

_NC_CACHE = None


def _build_nc():
    nblk = T_CORE // TOKBLK      # 4 blocks per core
    gp = TOKBLK // P             # 4 groups per block
    ngrp = T_CORE // P           # 16
    cs = NCHUNK // DMA_SPLIT

    nc = bacc.Bacc("TRN2", target_bir_lowering=False, debug=False)

    # hi/lo shards pre-laid-out on the host as the SBUF block image:
    # [blk][p][c*TOKBLK+t] = h[blk*TOKBLK+t, c*128+p] -> per-partition
    # contiguous DMA lines
    ht_hi = nc.dram_tensor(
        "ht_hi", [nblk, P, NCHUNK * TOKBLK], f16, kind="ExternalInput").ap()
    ht_lo = nc.dram_tensor(
        "ht_lo", [nblk, P, NCHUNK * TOKBLK], f16, kind="ExternalInput").ap()
    wt_hi = nc.dram_tensor("wt_hi", [D, E], f16, kind="ExternalInput").ap()
    wt_lo = nc.dram_tensor("wt_lo", [D, E], f16, kind="ExternalInput").ap()
    bias_hl = nc.dram_tensor("bias_hl", [1, 2 * E], f16, kind="ExternalInput").ap()
    ones = nc.dram_tensor("ones", [1, P], f16, kind="ExternalInput").ap()
    w_out = nc.dram_tensor("w_out", [P, ngrp, 2], f32, kind="ExternalOutput").ap()
    i_out = nc.dram_tensor("i_out", [P, ngrp, 2], i32, kind="ExternalOutput").ap()

    with tile.TileContext(nc) as tc, ExitStack() as ctx:
        const = ctx.enter_context(tc.tile_pool(name="const", bufs=1))
        wu_sb = const.tile([P, NCHUNK, E], f16)
        nc.sync.dma_start(wu_sb[:], wt_hi.rearrange("(c p) e -> p c e", p=P))
        wv_sb = const.tile([P, NCHUNK, E], f16)
        nc.sync.dma_start(wv_sb[:], wt_lo.rearrange("(c p) e -> p c e", p=P))
        bias_sb = const.tile([1, 2 * E], f16)
        nc.sync.dma_start(bias_sb[:], bias_hl)
        ones_sb = const.tile([1, P], f16)
        nc.sync.dma_start(ones_sb[:], ones)
        w_all = const.tile([P, ngrp, 2], f32)
        i_all = const.tile([P, ngrp, 2], i32)

        in_pool = ctx.enter_context(tc.tile_pool(name="hin", bufs=IN_BUFS))
        sm_pool = ctx.enter_context(tc.tile_pool(name="sm", bufs=3))
        mm_psum = ctx.enter_context(tc.tile_pool(name="mm_psum", bufs=6, space="PSUM"))

        for blk in range(nblk):
            t0 = blk * TOKBLK
            a_sb = in_pool.tile([P, NCHUNK, TOKBLK], f16, tag="ahin")
            b_sb = in_pool.tile([P, NCHUNK, TOKBLK], f16, tag="bhin")
            # lo-tensor DMAs issue from the Activation HWDGE queues so both
            # queue sets pull from HBM in parallel
            for piece in range(DMA_SPLIT):
                c0 = piece * cs
                fs = slice(c0 * TOKBLK, (c0 + cs) * TOKBLK)
                nc.sync.dma_start(a_sb[:, c0 : c0 + cs, :], ht_hi[blk][:, fs])
                nc.scalar.dma_start(b_sb[:, c0 : c0 + cs, :], ht_lo[blk][:, fs])

            for g in range(gp):
                grp = blk * gp + g
                gs = slice(g * P, (g + 1) * P)
                lg = mm_psum.tile([P, E], f32, tag="mm")
                # exact scaled bias preload: ones^T @ (32*(b+rb))_hi/lo
                nc.tensor.matmul(
                    lg[:], lhsT=ones_sb[:], rhs=bias_sb[:, 0:E],
                    start=True, stop=False,
                )
                nc.tensor.matmul(
                    lg[:], lhsT=ones_sb[:], rhs=bias_sb[:, E : 2 * E],
                    start=False, stop=False,
                )
                for c in range(NCHUNK):
                    nc.tensor.matmul(
                        lg[:], lhsT=a_sb[:, c, gs], rhs=wu_sb[:, c],
                        start=False, stop=False,
                    )
                    nc.tensor.matmul(
                        lg[:], lhsT=a_sb[:, c, gs], rhs=wv_sb[:, c],
                        start=False, stop=False,
                    )
                    nc.tensor.matmul(
                        lg[:], lhsT=b_sb[:, c, gs], rhs=wu_sb[:, c],
                        start=False, stop=(c == NCHUNK - 1),
                    )

                mx8 = sm_pool.tile([P, 8], f32, tag="mx8")
                nc.vector.max(out=mx8[:], in_=lg[:])
                idx8 = sm_pool.tile([P, 8], u32, tag="idx8")
                nc.vector.max_index(idx8[:], mx8[:], lg[:])
                nc.vector.tensor_copy(i_all[:, grp, :], idx8[:, 0:2])

                # scaled-logit softmax: exp(x/WSCALE + bias), so negm folds
                # the -max/WSCALE shift in one DVE op
                negm = sm_pool.tile([P, 1], f32, tag="negm")
                nc.vector.tensor_scalar_mul(negm[:], mx8[:, 0:1], -1.0 / WSCALE)
                escr = sm_pool.tile([P, E], f32, tag="escr")
                zsum = sm_pool.tile([P, 1], f32, tag="zsum")
                nc.scalar.activation(
                    escr[:],
                    lg[:],
                    mybir.ActivationFunctionType.Exp,
                    bias=negm[:],
                    scale=1.0 / WSCALE,
                    accum_out=zsum[:],
                )
                e2 = sm_pool.tile([P, 1], f32, tag="e2")
                nc.scalar.activation(
                    e2[:],
                    mx8[:, 1:2],
                    mybir.ActivationFunctionType.Exp,
                    bias=negm[:],
                    scale=1.0 / WSCALE,
                )
                zr = sm_pool.tile([P, 1], f32, tag="zr")
                nc.vector.reciprocal(zr[:], zsum[:])
                p2 = sm_pool.tile([P, 1], f32, tag="p2")
                nc.vector.tensor_mul(p2[:], e2[:], zr[:])
                s = sm_pool.tile([P, 1], f32, tag="s")
                nc.vector.tensor_add(s[:], zr[:], p2[:])
                nc.vector.tensor_scalar_add(s[:], s[:], EPS)
                sr = sm_pool.tile([P, 1], f32, tag="sr")
                nc.vector.reciprocal(sr[:], s[:])
                nc.vector.tensor_mul(w_all[:, grp, 0:1], zr[:], sr[:])
                nc.vector.tensor_mul(w_all[:, grp, 1:2], p2[:], sr[:])

            gsl = slice(blk * gp, (blk + 1) * gp)
            nc.sync.dma_start(w_out[:, gsl, :], w_all[:, gsl, :])
            nc.sync.dma_start(i_out[:, gsl, :], i_all[:, gsl, :])

    nc.compile()
    return nc


def _get_nc():
    global _NC_CACHE
    if _NC_CACHE is None:
        _NC_CACHE = _build_nc()
    return _NC_CACHE


def _split_fp16(x):
    hi = x.astype(np.float16)
    lo = (x - hi.astype(np.float32)).astype(np.float16)
    return hi, lo


def kernel(hidden_states, routing_bias, W, b):
    h2 = np.ascontiguousarray(np.asarray(hidden_states, dtype=np.float32)).reshape(
        T_TOTAL, D
    )
    wt32 = np.asarray(W, dtype=np.float32).T * WSCALE
    wt_hi, wt_lo = _split_fp16(wt32)
    wt_hi = np.ascontiguousarray(wt_hi)
    wt_lo = np.ascontiguousarray(wt_lo)
    bnp = np.asarray(b, dtype=np.float32)
    rb = np.asarray(routing_bias, dtype=np.float32)
    ones = np.ones((1, P), dtype=np.float16)

    nblk = T_CORE // TOKBLK

    def _blocked(x):
        # SBUF block image: [blk, p, c*TOKBLK+t] = h[blk*TOKBLK+t, c*128+p]
        return np.ascontiguousarray(
            x.reshape(nblk, TOKBLK, NCHUNK, P).transpose(0, 3, 2, 1)
        ).reshape(nblk, P, NCHUNK * TOKBLK)

    in_maps = []
    for c in range(N_CORES):
        t0 = c * T_CORE
        batch = t0 // S  # each core's token range lies within one batch
        h_hi, h_lo = _split_fp16(h2[t0 : t0 + T_CORE])
        bias32 = ((bnp + rb[batch]) * WSCALE).astype(np.float32)
        b_hi, b_lo = _split_fp16(bias32)
        in_maps.append(
            {
                "ht_hi": _blocked(h_hi),
                "ht_lo": _blocked(h_lo),
                "wt_hi": wt_hi,
                "wt_lo": wt_lo,
                "bias_hl": np.concatenate([b_hi, b_lo]).reshape(1, 2 * E),
                "ones": ones,
            }
        )

    nc = _get_nc()
    try:
        res = run_bass_kernel_spmd(nc, in_maps, list(range(N_CORES)))
    except Exception:
        # transient NRT/device hiccups have been observed to clear on retry
        res = run_bass_kernel_spmd(nc, in_maps, list(range(N_CORES)))

    ws, idxs = [], []
    for r in res.results:
        # [P, ngrp, 2] with token = grp*128 + partition
        ws.append(np.asarray(r["w_out"]).transpose(1, 0, 2).reshape(T_CORE, 2))
        idxs.append(np.asarray(r["i_out"]).transpose(1, 0, 2).reshape(T_CORE, 2))
    weights = np.concatenate(ws).reshape(B_DIM, S, 2).astype(np.float32)
    indices = np.concatenate(idxs).reshape(B_DIM, S, 2).astype(np.int32)
    return weights, indices
